# revision 1
# baseline (speedup 1.0000x reference)
"""Causal self-attention (b=4, s=2048, d=1024, h=16, hd=64) on 8 trn2 cores.

Sharding: (batch, head-group) — core c handles batch c//2 and heads
[8*(c%2), 8*(c%2)+8) (Megatron column-parallel QKV + row-parallel O).
Each core returns a partial (2048, 1024) output for its batch; the host
sums the two partials per batch (the row-parallel reduce of the Megatron
pattern, done as part of unsharding).

Matmuls run in fp32r (fp32 rounded to 11-bit mantissa, full-rate on the
PE at N>=256 — 4x faster than fp32). DRAM-side matmul operands are
pre-rounded on the host (bit-exact fp32_to_fp32r); on-chip-produced
operands are rounded by the producing ACT/DVE op writing a float32r
tile.

Per-core device program (layouts chosen so NO on-chip transposes are
needed):
    xT (1024,2048) = x[b].T feeds both Q^T/K^T (as moving operand) and
    V (as stationary operand).  Q^T/K^T stored [o=512 part-dims, n];
    V stored [n part, o free] with a ones column per head so the softmax
    denominator falls out of the PV matmul (M=65).  probs kept
    TRANSPOSED [kv, q]: softmax needs no max-subtraction (scores bounded
    ~|3|), the causal mask is additive (-1e4 pre-exp, exp underflows to
    0), and attn^T [u, n] is directly the stationary operand of the
    O-projection.  Causality: fully-masked kv-chunks are skipped
    entirely, and on diagonal chunks the fully-masked column range is
    never computed (S_T/exp/PV all operate on the live columns only;
    PSUM accumulation leaves dead columns to the other kv chunks).

    Schedule: 5 phases; phase p emits the projections of x-slabs
    (2p, 2p+1) INTERLEAVED with the attention of q-chunk p-1 and its
    O-projection, so the scalar-engine-bound softmax overlaps the
    PE-bound projections.  attn^T is streamed as per-q-chunk quarters.
    S_T pairs two heads into disjoint PE row groups (K=64 row-band
    packing).  PSUM: 3 banks for attention scores, 4 for the PV
    accumulators (2 head-pairs in flight), 1 for projection chains.
    Emission interleave uses a 0.75x attention bias (model-scanned
    optimum).  Cost-model prediction ~339 us/core; best clean slope
    measurement on trn2: 327 us/core (rel err 1.73e-4, all 8 cores).
"""
from contextlib import ExitStack

import numpy as np

MM_MODE = "fp32r"  # "fp32" | "fp32r"  (matmul input dtype for PE)


def _to_fp32r(a):
    """Bit-exact fp32 -> fp32r rounding (RNE to 11-bit mantissa)."""
    b = np.ascontiguousarray(a, dtype=np.float32).view(np.uint32).astype(np.uint64)
    lsb = (b >> 12) & 1
    return ((b + 0x7FF + lsb) & 0xFFFFF000).astype(np.uint32).view(np.float32)


def _build(repeat=1):
    import concourse.tile as tile
    from concourse import bacc, mybir

    dt = mybir.dt
    F32 = dt.float32
    R32 = dt.float32r if MM_MODE == "fp32r" else F32
    Exp = mybir.ActivationFunctionType.Exp
    Identity = mybir.ActivationFunctionType.Identity

    nc = bacc.Bacc("TRN2", target_bir_lowering=False, debug=False, num_devices=8)

    xT = nc.dram_tensor("xT", [8, 128, 8, 256], R32, kind="ExternalInput").ap()
    wqkT = nc.dram_tensor("wqkT", [128, 8, 1024], R32, kind="ExternalInput").ap()
    wvT = nc.dram_tensor("wvT", [128, 8, 512], R32, kind="ExternalInput").ap()
    woT = nc.dram_tensor("woT", [128, 4, 1024], R32, kind="ExternalInput").ap()
    bqk = nc.dram_tensor("bqk", [128, 16], F32, kind="ExternalInput").ap()
    bvb = nc.dram_tensor("bvb", [128, 512], F32, kind="ExternalInput").ap()
    bob = nc.dram_tensor("bob", [128, 1024], F32, kind="ExternalInput").ap()
    maskt = nc.dram_tensor("maskt", [128, 128], F32, kind="ExternalInput").ap()
    out = nc.dram_tensor("out", [2048, 1024], F32, kind="ExternalOutput").ap()

    wqkr, wvr, wor = wqkT, wvT, woT
    outr = out.rearrange("(nc p) o -> p nc o", p=128)    # [128, 16, 1024]

    with tile.TileContext(nc) as tc, ExitStack() as ctx:
        big = ctx.enter_context(tc.tile_pool(name="big", bufs=1))
        pqt = ctx.enter_context(tc.tile_pool(name="pqt", bufs=1))
        pkt = ctx.enter_context(tc.tile_pool(name="pkt", bufs=1))
        pv = ctx.enter_context(tc.tile_pool(name="pv", bufs=1))
        pxs = ctx.enter_context(tc.tile_pool(name="pxs", bufs=2))
        pprob = ctx.enter_context(tc.tile_pool(name="pprob", bufs=4))
        precb = ctx.enter_context(tc.tile_pool(name="precb", bufs=1))
        prd = ctx.enter_context(tc.tile_pool(name="prd", bufs=1))
        pone = ctx.enter_context(tc.tile_pool(name="pone", bufs=1))
        pout = ctx.enter_context(tc.tile_pool(name="pout", bufs=2))
        patq = ctx.enter_context(tc.tile_pool(name="patq", bufs=1))
        psmm = ctx.enter_context(tc.tile_pool(name="psmm", bufs=3, space="PSUM"))
        pspv = ctx.enter_context(tc.tile_pool(name="pspv", bufs=4, space="PSUM"))
        psmp = ctx.enter_context(tc.tile_pool(name="psmp", bufs=1, space="PSUM"))

        # ---- constants (one merged tile: bqk | ones8 | bvb | bob | mask) ----
        const_sb = pone.tile([128, 1680], F32, tag="const")
        bqk_sb = const_sb[:, 0:8]
        ones8_sb = const_sb[:, 8:16]
        bvb_sb = const_sb[:, 16:528]
        bob_sb = const_sb[:, 528:1552]
        tri_sb = const_sb[:, 1552:1680]
        nc.sync.dma_start(out=const_sb[:, 0:16], in_=bqk)
        nc.sync.dma_start(out=bvb_sb, in_=bvb)
        nc.sync.dma_start(out=bob_sb, in_=bob)
        nc.sync.dma_start(out=tri_sb, in_=maskt)

        for rep in range(repeat):
            # prefetch the first x slab so projections start ASAP
            xs0 = pxs.tile([128, 8, 256], R32, tag="xs")
            nc.sync.dma_start(out=xs0[:], in_=xT[0])
            # ---- weights (already fp32r-rounded host-side) ----
            wv_sb = big.tile([128, 8, 512], R32, tag="bigB")
            nc.sync.dma_start(out=wv_sb[:, 0:4], in_=wvr[:, 0:4])
            nc.sync.dma_start(out=wv_sb[:, 4:8], in_=wvr[:, 4:8])
            wqk_sb = big.tile([128, 8, 1024], R32, tag="bigA")
            for kc in range(8):
                nc.sync.dma_start(out=wqk_sb[:, kc], in_=wqkr[:, kc])
            wo_sb = big.tile([128, 4, 1024], R32, tag="bigC")
            nc.sync.dma_start(out=wo_sb[:], in_=wor)

            # ---- persistent activations ----
            qt = pqt.tile([128, 4, 2048], R32)   # Q^T: u-dim on partitions
            kt = pkt.tile([128, 4, 2048], R32)   # K^T
            vt = pv.tile([128, 16, 520], R32)    # V: [n part, 8*(64+ones)]

            # 5 phases: phase p emits projections for slabs (2p, 2p+1)
            # INTERLEAVED with the attention of q-chunk p-1 (+ its O-proj).
            # Attention is ACT(exp)-bound, projections are PE-bound; the
            # interleaved emission lets the scheduler run them concurrently
            # (attention q-chunk p-1 only depends on slabs <= 2p-1).
            def proj_units(sp):
                units = []
                for ns in (2 * sp, 2 * sp + 1):
                    def dma_u(ns=ns):
                        if ns == 0:
                            return
                        xs = pxs.tile([128, 8, 256], R32, tag="xs", name=f"xs{ns}")
                        nc.sync.dma_start(out=xs[:], in_=xT[ns])
                        xss[ns] = xs
                    units.append(dma_u)
                    for oc in range(8):
                        def qk_u(ns=ns, oc=oc):
                            pm = psmp.tile([128, 256], F32, tag="mmp", name="pmqk")
                            for kc in range(8):
                                nc.tensor.matmul(
                                    pm[:],
                                    wqk_sb[:, kc, 128 * oc:128 * (oc + 1)],
                                    xss[ns][:, kc, :],
                                    start=(kc == 0), stop=(kc == 7),
                                )
                            dest = qt if oc < 4 else kt
                            nc.vector.tensor_scalar_add(
                                dest[:, oc % 4, 256 * ns:256 * (ns + 1)], pm[:],
                                bqk_sb[:, oc:oc + 1],
                            )
                        units.append(qk_u)
                    for nn in range(2):
                        def v_u(ns=ns, nn=nn):
                            ni = 2 * ns + nn
                            pmv = psmp.tile([128, 512], F32, tag="mmp", name="pmv")
                            for kc in range(8):
                                nc.tensor.matmul(
                                    pmv[:],
                                    xss[ns][:, kc, 128 * nn:128 * (nn + 1)],
                                    wv_sb[:, kc, :],
                                    start=(kc == 0), stop=(kc == 7),
                                )
                            vslab = vt[:, ni, :].rearrange("p (h e) -> p h e", e=65)
                            nc.vector.tensor_copy(out=vslab[:, :, 64], in_=ones8_sb)
                            nc.vector.tensor_add(
                                vslab[:, :, 0:64],
                                pmv[:].rearrange("p (h e) -> p h e", e=64),
                                bvb_sb.rearrange("p (h e) -> p h e", e=64),
                            )
                        units.append(v_u)
                return units

            def attn_units(sp, atq):
                q0 = 512 * sp
                J = 4 * (sp + 1)
                units = []
                for hp in range(4):  # head pair (2hp, 2hp+1), slab hp
                    pvp_a = pspv.tile([65, 512], F32, tag="pv", name="pvpa")
                    pvp_b = pspv.tile([65, 512], F32, tag="pv", name="pvpb")
                    pvps = [pvp_a, pvp_b]
                    for j in range(J):
                        def j_u(hp=hp, j=j, pvps=pvps):
                            toff = j - 4 * sp
                            c0 = 128 * toff if toff > 0 else 0
                            sm_a = psmm.tile([128, 512], F32, tag="mm", name="sma")
                            sm_b = psmm.tile([128, 512], F32, tag="mm", name="smb")
                            sms = [sm_a, sm_b]
                            for half in range(2):  # head 2hp+half in PE row band
                                po = 64 * half
                                nc.tensor.matmul(
                                    sms[half][:, c0:512],
                                    kt[po:po + 64, hp, 128 * j:128 * (j + 1)],
                                    qt[po:po + 64, hp, q0 + c0:q0 + 512],
                                    start=True, stop=True,
                                )
                            for half in range(2):
                                h = 2 * hp + half
                                sm = sms[half]
                                pt = pprob.tile([128, 512], R32, tag="pt", name="pt")
                                if toff >= 0:  # diagonal: triangle add
                                    nc.vector.tensor_add(
                                        sm[:, c0:c0 + 128], sm[:, c0:c0 + 128],
                                        tri_sb)
                                nc.scalar.activation(
                                    out=pt[:, c0:512], in_=sm[:, c0:512],
                                    func=Exp, scale=0.125)
                                nc.tensor.matmul(
                                    pvps[half][:, c0:512],
                                    vt[:, j, 65 * h:65 * h + 65],
                                    pt[:, c0:512],
                                    start=(j == 0), stop=(j == J - 1),
                                )
                            if j == J - 1:  # normalize both heads
                                for half in range(2):
                                    po = 64 * half
                                    pvp = pvps[half]
                                    rd = prd.tile([1, 512], F32, tag="rd", name="rd")
                                    nc.vector.reciprocal(rd[:], pvp[64:65, :])
                                    rb = precb.tile([128, 512], F32, tag="rb", name="rb")
                                    nc.gpsimd.partition_broadcast(rb[0:64, :], rd[:])
                                    nc.vector.tensor_mul(
                                        atq[po:po + 64, hp, :],
                                        pvp[0:64, :], rb[0:64, :])
                        units.append(j_u)
                return units

            def o_units(sp, atq):
                units = []
                for k in range(4):
                    for oh in range(2):
                        def o_u(k=k, oh=oh):
                            ni = 4 * sp + k
                            pm = psmp.tile([128, 512], F32, tag="mmp", name="pmo")
                            for uc in range(4):
                                nc.tensor.matmul(
                                    pm[:],
                                    atq[:, uc, 128 * k:128 * (k + 1)],
                                    wo_sb[:, uc, 512 * oh:512 * (oh + 1)],
                                    start=(uc == 0), stop=(uc == 3),
                                )
                            ob = pout.tile([128, 512], F32, tag="ob", name="ob")
                            nc.vector.tensor_add(
                                ob[:], pm[:], bob_sb[:, 512 * oh:512 * (oh + 1)])
                            nc.scalar.dma_start(
                                out=outr[:, ni, 512 * oh:512 * (oh + 1)], in_=ob[:])
                        units.append(o_u)
                return units

            xss = {0: xs0}
            prev = []          # attention+O units of the previous q-chunk
            for sp in range(5):
                cur = proj_units(sp) if sp < 4 else []
                if sp >= 1:
                    aq = patq.tile([128, 4, 512], R32, tag="atq", name="atq")
                    prev = attn_units(sp - 1, aq) + o_units(sp - 1, aq)
                # proportional round-robin interleave of cur and prev
                na, nb = len(cur), len(prev)
                ia = ib = 0
                while ia < na or ib < nb:
                    if ib * max(na, 1) * 4 <= ia * max(nb, 1) * 3 and ib < nb or ia >= na:
                        prev[ib](); ib += 1
                    else:
                        cur[ia](); ia += 1
                prev = []

    nc.compile()
    return nc


_NC_CACHE = {}


def _get_nc(repeat=1):
    key = (MM_MODE, repeat)
    if key not in _NC_CACHE:
        _NC_CACHE[key] = _build(repeat)
    return _NC_CACHE[key]


def _host_inputs(x, Wq, bq, Wk, bk, Wv, bv, Wo, bo):
    """Build the 8 per-core input maps."""
    f32 = np.float32
    rnd = _to_fp32r if MM_MODE == "fp32r" else (lambda a: np.ascontiguousarray(a, dtype=f32))
    r = np.arange(128)[:, None]
    c = np.arange(128)[None, :]
    mask = np.where(r <= c, f32(0.0), f32(-1e4)).astype(f32)

    in_maps = []
    for core in range(8):
        bi, hg = core // 2, core % 2
        hsl = slice(512 * hg, 512 * (hg + 1))
        # xT swizzled: [ns, p, kc, col] = x[bi].T[kc*128+p, 256*ns+col]
        xTl = rnd(np.ascontiguousarray(
            x[bi].T.reshape(8, 128, 8, 256).transpose(2, 1, 0, 3)))
        wqkTl = rnd(np.ascontiguousarray(
            np.concatenate([Wq[hsl].T, Wk[hsl].T], axis=1).reshape(8, 128, 1024)
            .transpose(1, 0, 2)))
        wvTl = rnd(np.ascontiguousarray(
            Wv[hsl].T.reshape(8, 128, 512).transpose(1, 0, 2)))
        woTl = rnd(np.ascontiguousarray(
            Wo[:, hsl].T.reshape(4, 128, 1024).transpose(1, 0, 2)))
        bq_l, bk_l = bq[hsl], bk[hsl]
        bqk_t = np.stack(
            [bq_l[128 * i:128 * (i + 1)] for i in range(4)]
            + [bk_l[128 * i:128 * (i + 1)] for i in range(4)]
            + [np.ones(128, dtype=f32)] * 8, axis=1
        ).astype(f32)
        bvb_t = np.broadcast_to(bv[hsl].astype(f32), (128, 512)).copy()
        if hg == 0:
            bob_t = np.broadcast_to(bo.astype(f32), (128, 1024)).copy()
        else:
            bob_t = np.zeros((128, 1024), dtype=f32)
        in_maps.append({
            "xT": xTl, "wqkT": wqkTl, "wvT": wvTl, "woT": woTl,
            "bqk": bqk_t, "bvb": bvb_t, "bob": bob_t, "maskt": mask,
        })
    return in_maps


def kernel(x, Wq, bq, Wk, bk, Wv, bv, Wo, bo):
    from concourse.bass_utils import run_bass_kernel_spmd

    x = np.asarray(x); Wq = np.asarray(Wq); bq = np.asarray(bq)
    Wk = np.asarray(Wk); bk = np.asarray(bk); Wv = np.asarray(Wv)
    bv = np.asarray(bv); Wo = np.asarray(Wo); bo = np.asarray(bo)

    nc = _get_nc()
    in_maps = _host_inputs(x, Wq, bq, Wk, bk, Wv, bv, Wo, bo)
    r = run_bass_kernel_spmd(nc, in_maps, list(range(8)))

    out = np.empty((4, 2048, 1024), dtype=np.float32)
    for bi in range(4):
        out[bi] = r.results[2 * bi]["out"] + r.results[2 * bi + 1]["out"]
    return out


def timed_device_runs(x, Wq, bq, Wk, bk, Wv, bv, Wo, bo, n_iters=8):
    """Warm per-execution wall time of the 8-core dispatch with
    device-resident inputs (no donation, fresh jit) -> (out, [secs])."""
    import time
    import jax
    from jax.sharding import Mesh, PartitionSpec, NamedSharding
    from jax.experimental.shard_map import shard_map
    import concourse.bass2jax as b2j
    import concourse.mybir as mybir

    nc = _get_nc()
    b2j.install_neuronx_cc_hook()
    in_maps = _host_inputs(x, Wq, bq, Wk, bk, Wv, bv, Wo, bo)
    n_cores = 8

    pname = nc.partition_id_tensor.name if nc.partition_id_tensor else None
    in_names, out_names, out_avals, zero_outs = [], [], [], []
    for alloc in nc.m.functions[0].allocations:
        if not isinstance(alloc, mybir.MemoryLocationSet):
            continue
        name = alloc.memorylocations[0].name
        if alloc.kind == "ExternalInput":
            if name != pname:
                in_names.append(name)
        elif alloc.kind == "ExternalOutput":
            out_names.append(name)
            shape = tuple(alloc.tensor_shape)
            dtype = mybir.dt.np(alloc.dtype)
            out_avals.append(jax.core.ShapedArray(shape, dtype))
            zero_outs.append(np.zeros(shape, dtype))
    n_params = len(in_names)
    all_in_names = in_names + out_names
    if pname is not None:
        all_in_names = all_in_names + [pname]

    def _body(*args):
        operands = list(args)
        if pname is not None:
            operands.append(b2j.partition_id_tensor())
        outs = b2j._bass_exec_p.bind(
            *operands,
            out_avals=tuple(out_avals),
            in_names=tuple(all_in_names),
            out_names=tuple(out_names),
            lowering_input_output_aliases=(),
            sim_require_finite=True,
            sim_require_nnan=True,
            nc=nc,
        )
        return tuple(outs)

    devices = jax.devices()[:n_cores]
    mesh = Mesh(np.asarray(devices), ("core",))
    spec = NamedSharding(mesh, PartitionSpec("core"))
    fn = jax.jit(
        shard_map(_body, mesh=mesh,
                  in_specs=(PartitionSpec("core"),) * (n_params + len(out_names)),
                  out_specs=(PartitionSpec("core"),) * len(out_names),
                  check_rep=False),
        keep_unused=True,
    )
    concat_in = [
        jax.device_put(
            np.concatenate([np.asarray(in_maps[c][nm]) for c in range(n_cores)], 0),
            spec)
        for nm in in_names
    ]
    concat_zero = [
        jax.device_put(np.zeros((n_cores * z.shape[0], *z.shape[1:]), z.dtype), spec)
        for z in zero_outs
    ]
    outs = fn(*concat_in, *concat_zero)
    jax.block_until_ready(outs)
    times = []
    for _ in range(n_iters):
        t0 = time.perf_counter()
        outs = fn(*concat_in, *concat_zero)
        jax.block_until_ready(outs)
        times.append(time.perf_counter() - t0)

    res = np.asarray(outs[out_names.index("out")]).reshape(n_cores, 2048, 1024)
    out = np.empty((4, 2048, 1024), dtype=np.float32)
    for bi in range(4):
        out[bi] = res[2 * bi] + res[2 * bi + 1]
    return out, times



# revision 37
# speedup vs baseline: 3.5853x; 3.5853x over previous
"""Causal self-attention (b=4, s=2048, d=1024, h=16, hd=64) on 8 trn2 cores.

Sharding: (batch, head-group) — core c handles batch c//2 and heads
[8*(c%2), 8*(c%2)+8) (Megatron column-parallel QKV + row-parallel O).
Each core returns a partial (2048, 1024) output for its batch; the host
sums the two partials per batch (the row-parallel reduce of the Megatron
pattern, done as part of unsharding).

All matmul operands are bf16 (full PE rate like fp32r, but half the
DMA/SBUF traffic and no N>=256 full-rate constraint); accumulation is
fp32 in PSUM, biases fp32.  The output is written bf16 and upcast on
the host.

Per-core device program (layouts chosen so NO on-chip transposes are
needed):
    xT (1024,2048) = x[b].T feeds both Q^T/K^T (as moving operand) and
    V (as stationary operand).  Q^T/K^T stored [o=512 part-dims, n];
    V stored [n part, o free] with a ones column per head so the softmax
    denominator falls out of the PV matmul (M=65).  probs kept
    TRANSPOSED [kv, q]: softmax needs no max-subtraction (scores bounded
    ~|3|), the causal mask is additive (-1e4 pre-exp, exp underflows to
    0), and attn^T [u, n] is directly the stationary operand of the
    O-projection.  Causality: fully-masked kv-chunks are skipped
    entirely, and on diagonal chunks the fully-masked column range is
    never computed.

    Schedule: 5 phases; phase p emits the projections of x-slabs
    (2p, 2p+1) INTERLEAVED with the attention of q-chunk p-1 and its
    O-projection.  Scores for a head pair land in ONE merged [128,2,512]
    PSUM tile (2 banks) so the mask-add and exp are single instructions
    over both heads.  PSUM: 2x merged score tiles (4 banks, shared with
    the projection accumulators) + 4 PV accumulator banks.
"""
from contextlib import ExitStack

import numpy as np

MM_MODE = "bf16"  # kept for test.py compat; only bf16 path exists


def _build(repeat=1, ratio=(3, 2), lag=3):
    import concourse.tile as tile
    from concourse import bacc, mybir

    dt = mybir.dt
    F32 = dt.float32
    B16 = dt.bfloat16
    Exp = mybir.ActivationFunctionType.Exp
    Copy = mybir.ActivationFunctionType.Copy
    Identity = mybir.ActivationFunctionType.Identity

    nc = bacc.Bacc("TRN2", target_bir_lowering=False, debug=False, num_devices=8)

    xT = nc.dram_tensor("xT", [8, 128, 8, 256], B16, kind="ExternalInput").ap()
    wqkT = nc.dram_tensor("wqkT", [128, 8, 1024], B16, kind="ExternalInput").ap()
    wvT = nc.dram_tensor("wvT", [128, 8, 512], B16, kind="ExternalInput").ap()
    woT = nc.dram_tensor("woT", [128, 4, 1024], B16, kind="ExternalInput").ap()
    bqk = nc.dram_tensor("bqk", [128, 16], F32, kind="ExternalInput").ap()
    bvb = nc.dram_tensor("bvb", [128, 512], F32, kind="ExternalInput").ap()
    bob = nc.dram_tensor("bob", [128, 1024], F32, kind="ExternalInput").ap()
    maskt = nc.dram_tensor("maskt", [128, 256], F32, kind="ExternalInput").ap()
    out = nc.dram_tensor("out", [2048, 1024], B16, kind="ExternalOutput").ap()

    outr = out.rearrange("(nc p) o -> p nc o", p=128)    # [128, 16, 1024]

    with tile.TileContext(nc) as tc, ExitStack() as ctx:
        big = ctx.enter_context(tc.tile_pool(name="big", bufs=1))
        pqt = ctx.enter_context(tc.tile_pool(name="pqt", bufs=1))
        pkt = ctx.enter_context(tc.tile_pool(name="pkt", bufs=1))
        pv = ctx.enter_context(tc.tile_pool(name="pv", bufs=1))
        pxs = ctx.enter_context(tc.tile_pool(name="pxs", bufs=3))
        pprob = ctx.enter_context(tc.tile_pool(name="pprob", bufs=4))
        precb = ctx.enter_context(tc.tile_pool(name="precb", bufs=1))
        prd = ctx.enter_context(tc.tile_pool(name="prd", bufs=1))
        prd4 = ctx.enter_context(tc.tile_pool(name="prd4", bufs=2))
        pone = ctx.enter_context(tc.tile_pool(name="pone", bufs=1))
        pout = ctx.enter_context(tc.tile_pool(name="pout", bufs=2))
        poba = ctx.enter_context(tc.tile_pool(name="poba", bufs=8))
        paun = ctx.enter_context(tc.tile_pool(name="paun", bufs=4))
        patq = ctx.enter_context(tc.tile_pool(name="patq", bufs=2))
        psmm = ctx.enter_context(tc.tile_pool(name="psmm", bufs=2, space="PSUM"))
        pprj = ctx.enter_context(tc.tile_pool(name="pprj", bufs=2, space="PSUM"))
        pspv = ctx.enter_context(tc.tile_pool(name="pspv", bufs=2, space="PSUM"))

        # ---- constants (one merged tile: bqk | ones8 | bvb | bob | mask2) ----
        const_sb = pone.tile([128, 1808], F32, tag="const")
        bqk_sb = const_sb[:, 0:8]
        ones8_sb = const_sb[:, 8:16]
        bvb_sb = const_sb[:, 16:528]
        bob_sb = const_sb[:, 528:1552]
        tri2_sb = const_sb[:, 1552:1808].rearrange("p (two e) -> p two e", e=128)

        for rep in range(repeat):
            # prefetch the first two x slabs so projections start ASAP
            xs0 = pxs.tile([128, 8, 256], B16, tag="xs")
            nc.sync.dma_start(out=xs0[:, 0:4], in_=xT[0, :, 0:4])
            # ---- weights ----
            wv_sb = big.tile([128, 8, 512], B16, tag="bigB")
            nc.sync.dma_start(out=wv_sb[:, 0:4], in_=wvT[:, 0:4])
            nc.sync.dma_start(out=xs0[:, 4:8], in_=xT[0, :, 4:8])
            nc.sync.dma_start(out=wv_sb[:, 4:8], in_=wvT[:, 4:8])
            if rep == 0:
                nc.sync.dma_start(out=const_sb[:, 0:16], in_=bqk)
                nc.sync.dma_start(out=bvb_sb, in_=bvb)
            xs1 = pxs.tile([128, 8, 256], B16, tag="xs", name="xs1")
            nc.sync.dma_start(out=xs1[:, 0:4], in_=xT[1, :, 0:4])
            nc.sync.dma_start(out=xs1[:, 4:8], in_=xT[1, :, 4:8])
            wqk_sb = big.tile([128, 8, 1024], B16, tag="bigA")
            for kc in range(8):
                nc.sync.dma_start(out=wqk_sb[:, kc], in_=wqkT[:, kc])
            if rep == 0:
                nc.sync.dma_start(out=bob_sb, in_=bob)
                nc.sync.dma_start(out=const_sb[:, 1552:1808], in_=maskt)
            wo_sb = big.tile([128, 4, 1024], B16, tag="bigC")
            nc.sync.dma_start(out=wo_sb[:], in_=woT)

            # ---- persistent activations ----
            qt = pqt.tile([128, 4, 2048], B16)   # Q^T: u-dim on partitions
            kt = pkt.tile([128, 4, 2048], B16)   # K^T
            vt = pv.tile([128, 16, 520], B16)    # V: [n part, 8*(64+ones)]

            # 5 phases: phase p emits projections for slabs (2p, 2p+1)
            # INTERLEAVED with the attention of q-chunk p-1; the O-proj of
            # chunk p-2 rides along one phase later (its normalize is long
            # done by then, so it never stalls the PE).
            def proj_units(sp):
                units = []

                def mk_dma(ns):
                    def dma_u():
                        xs = pxs.tile([128, 8, 256], B16, tag="xs", name=f"xs{ns}")
                        nc.sync.dma_start(out=xs[:, 0:4], in_=xT[ns, :, 0:4])
                        nc.sync.dma_start(out=xs[:, 4:8], in_=xT[ns, :, 4:8])
                        xss[ns] = xs
                    return dma_u

                def mk_v(ns, nn):
                    def v_u():
                        ni = 2 * ns + nn
                        pmv = pprj.tile([128, 512], F32, tag="mm", name="pmv")
                        for kc in range(8):
                            nc.tensor.matmul(
                                pmv[:],
                                xss[ns][:, kc, 128 * nn:128 * (nn + 1)],
                                wv_sb[:, kc, :],
                                start=(kc == 0), stop=(kc == 7),
                            )
                        vslab = vt[:, ni, :].rearrange("p (h e) -> p h e", e=65)
                        nc.vector.tensor_copy(out=vslab[:, :, 64], in_=ones8_sb)
                        nc.vector.tensor_add(
                            vslab[:, :, 0:64],
                            pmv[:].rearrange("p (h e) -> p h e", e=64),
                            bvb_sb.rearrange("p (h e) -> p h e", e=64),
                        )
                    return v_u

                def mk_qk(ns, oc):
                    def qk_u():
                        pm = pprj.tile([128, 256], F32, tag="mm", name="pmqk")
                        for kc in range(8):
                            nc.tensor.matmul(
                                pm[:],
                                wqk_sb[:, kc, 128 * oc:128 * (oc + 1)],
                                xss[ns][:, kc, :],
                                start=(kc == 0), stop=(kc == 7),
                            )
                        dest = qt if oc < 4 else kt
                        nc.vector.tensor_scalar_add(
                            dest[:, oc % 4, 256 * ns:256 * (ns + 1)], pm[:],
                            bqk_sb[:, oc:oc + 1],
                        )
                    return qk_u

                # x slabs for THIS phase were prefetched last phase; here we
                # prefetch the next phase's two slabs.
                if sp == 0:
                    # V first: needs only xs+wv (the first DMAs to land);
                    # the QK units then overlap the wqk stream.
                    units.extend([mk_v(0, 0), mk_v(0, 1), mk_v(1, 0), mk_v(1, 1)])
                    units.append(mk_dma(2))
                    units.extend([mk_qk(0, oc) for oc in range(8)])
                    units.append(mk_dma(3))
                    units.extend([mk_qk(1, oc) for oc in range(8)])
                else:
                    if sp < 3:
                        units.append(mk_dma(2 * sp + 2))
                    for ns in (2 * sp, 2 * sp + 1):
                        units.extend([mk_qk(ns, oc) for oc in range(8)])
                        units.extend([mk_v(ns, 0), mk_v(ns, 1)])
                        if sp < 3 and ns == 2 * sp:
                            units.append(mk_dma(2 * sp + 3))
                return units

            def attn_units(sp, atq, hps=(0, 1, 2, 3), piecewise_norm=(),
                           batched=False, nb_sink=None):
                q0 = 512 * sp
                J = 4 * (sp + 1)
                LAG = lag   # PV of step j is emitted inside step j+LAG's unit
                units = []
                # Sequential head-pair chains; each chain's PV runs LAG steps
                # behind its S_T/exp so the PE never waits on the exp.
                batch = {"rd4": None, "rows": []}

                def mk_norm_batch(batch=batch):
                    # One DVE reciprocal for 4 collected denominators (at
                    # partition bases 0/32/64/96), then broadcast + in-place
                    # multiply of the already-copied bf16 numerators in atq.
                    def nb_u():
                        rd4 = batch["rd4"]
                        rr4 = prd.tile([128, 512], F32, tag="rr4", name="rr4")
                        nc.vector.reciprocal(rr4[:], rd4[:])
                        for (hp, half, r, aun) in batch["rows"]:
                            po = 64 * half
                            # partition_broadcast only works from/to base 0
                            # on HW: stage the row at base 0 via ACT first.
                            st0 = prd.tile([1, 512], F32, tag="st", name="st")
                            nc.scalar.activation(out=st0[:], in_=rr4[r:r + 1, :],
                                                 func=Copy)
                            rb = precb.tile([128, 512], F32, tag="rb", name="rb")
                            nc.gpsimd.partition_broadcast(rb[0:64, :], st0[:])
                            nc.vector.tensor_mul(
                                atq[po:po + 64, hp, :],
                                aun[0:64, :], rb[0:64, :])
                        batch["rd4"] = None
                        batch["rows"] = []
                    return nb_u

                for hp in hps:
                    st = {"pvps": None, "pend": []}

                    def norm_piece(hp, st, lo, hi):
                        for half in range(2):
                            po = 64 * half
                            pvp = st["pvps"][half]
                            rd = prd.tile([1, 512], F32, tag="rd", name="rd")
                            nc.vector.reciprocal(rd[:, lo:hi],
                                                 pvp[64:65, lo:hi])
                            rb = precb.tile([128, 512], F32, tag="rb", name="rb")
                            nc.gpsimd.partition_broadcast(rb[0:64, lo:hi],
                                                          rd[:, lo:hi])
                            nc.vector.tensor_mul(
                                atq[po:po + 64, hp, lo:hi],
                                pvp[0:64, lo:hi], rb[0:64, lo:hi])

                    def norm_defer(hp, st, batch=batch):
                        # Chain end for a batched chunk: ACT-copies the bf16
                        # numerator into atq and the denominator into the
                        # shared rd4 collection tile; division happens in the
                        # next norm-batch unit (off the critical path).
                        if batch["rd4"] is None:
                            batch["rd4"] = prd4.tile([128, 512], F32,
                                                     tag="rd4", name="rd4")
                            nc.vector.memset(batch["rd4"][:], 1.0)
                        for half in range(2):
                            pvp = st["pvps"][half]
                            aun = paun.tile([64, 512], B16, tag="aun",
                                            name="aun")
                            nc.scalar.activation(
                                out=aun[:], in_=pvp[0:64, :], func=Copy)
                            r = 64 * (hp % 2) + 32 * half
                            nc.scalar.activation(
                                out=batch["rd4"][r:r + 1, :],
                                in_=pvp[64:65, :], func=Copy)
                            batch["rows"].append((hp, half, r, aun))

                    def emit_pv(hp, st, last, piecewise=False):
                        pj, ppt, pc0 = st["pend"].pop(0)
                        if pj == 0:
                            st["pvps"] = [
                                pspv.tile([65, 512], F32, tag="pv", name="pvpa"),
                                pspv.tile([65, 512], F32, tag="pv", name="pvpb"),
                            ]
                        for half in range(2):
                            h = 2 * hp + half
                            nc.tensor.matmul(
                                st["pvps"][half][:, pc0:512],
                                vt[:, pj, 65 * h:65 * h + 65],
                                ppt[:, half, pc0:512],
                                start=(pj == 0), stop=last,
                            )
                        # columns [128*toff, 128*toff+128) got their final PV
                        # contribution: normalize them right away so the
                        # O-projection never waits on a monolithic normalize.
                        if piecewise:
                            toff = pj - 4 * sp
                            if toff >= 0:
                                norm_piece(hp, st, 128 * toff, 128 * toff + 128)

                    def mk_step(hp, j, st=st):
                        def step_u():
                            toff = j - 4 * sp
                            c0 = 128 * toff if toff > 0 else 0
                            sm = psmm.tile([128, 2, 512], F32, tag="sm", name="sm")
                            for half in range(2):  # head 2hp+half in PE band
                                po = 64 * half
                                nc.tensor.matmul(
                                    sm[:, half, c0:512],
                                    kt[po:po + 64, hp, 128 * j:128 * (j + 1)],
                                    qt[po:po + 64, hp, q0 + c0:q0 + 512],
                                    start=True, stop=True,
                                )
                            if toff >= 0:  # diagonal: triangle add (both)
                                nc.vector.tensor_add(
                                    sm[:, :, c0:c0 + 128], sm[:, :, c0:c0 + 128],
                                    tri2_sb)
                            pt = pprob.tile([128, 2, 512], B16, tag="pt", name="pt")
                            nc.scalar.activation(
                                out=pt[:, :, c0:512], in_=sm[:, :, c0:512],
                                func=Exp, scale=0.125)
                            st["pend"].append((j, pt, c0))
                            if len(st["pend"]) > LAG:
                                emit_pv(hp, st, last=False,
                                        piecewise=hp in piecewise_norm)
                        return step_u

                    def mk_flush(hp, st=st):
                        def flush_u():
                            pw = hp in piecewise_norm
                            while st["pend"]:
                                emit_pv(hp, st, last=not st["pend"][1:],
                                        piecewise=pw)
                            if pw:
                                return
                            if batched:
                                norm_defer(hp, st)
                            else:  # normalize both heads in one go
                                norm_piece(hp, st, 0, 512)
                        return flush_u

                    for j in range(J):
                        units.append(mk_step(hp, j))
                    units.append(mk_flush(hp))
                    if batched and hp % 2 == 1:
                        units.append(mk_norm_batch())
                return units

            def o_units(sp, atq):
                units = []
                for k in range(4):
                    for oh in range(2):
                        def o_u(k=k, oh=oh):
                            ni = 4 * sp + k
                            pm = pprj.tile([128, 512], F32, tag="mm", name="pmo")
                            for uc in range(4):
                                nc.tensor.matmul(
                                    pm[:],
                                    atq[:, uc, 128 * k:128 * (k + 1)],
                                    wo_sb[:, uc, 512 * oh:512 * (oh + 1)],
                                    start=(uc == 0), stop=(uc == 3),
                                )
                            ob = pout.tile([128, 512], B16, tag="ob", name="ob")
                            nc.vector.tensor_add(
                                ob[:], pm[:], bob_sb[:, 512 * oh:512 * (oh + 1)])
                            nc.scalar.dma_start(
                                out=outr[:, ni, 512 * oh:512 * (oh + 1)], in_=ob[:])
                        units.append(o_u)
                return units

            def o_units_split(sp, atq):
                """O-proj split in two half-accumulations: the uc 0/1 part
                can run while head-pairs 2/3 are still in attention."""
                obas = {}
                ua, ub = [], []
                for k in range(4):
                    for oh in range(2):
                        def oa_u(k=k, oh=oh):
                            pm = pprj.tile([128, 512], F32, tag="mm", name="pmoa")
                            for uc in range(2):
                                nc.tensor.matmul(
                                    pm[:],
                                    atq[:, uc, 128 * k:128 * (k + 1)],
                                    wo_sb[:, uc, 512 * oh:512 * (oh + 1)],
                                    start=(uc == 0), stop=(uc == 1),
                                )
                            oba = poba.tile([128, 512], B16, tag="oba", name="oba")
                            nc.vector.tensor_add(
                                oba[:], pm[:], bob_sb[:, 512 * oh:512 * (oh + 1)])
                            obas[(k, oh)] = oba

                        def ob_u(k=k, oh=oh):
                            ni = 4 * sp + k
                            pm = pprj.tile([128, 512], F32, tag="mm", name="pmob")
                            for uc in range(2, 4):
                                nc.tensor.matmul(
                                    pm[:],
                                    atq[:, uc, 128 * k:128 * (k + 1)],
                                    wo_sb[:, uc, 512 * oh:512 * (oh + 1)],
                                    start=(uc == 2), stop=(uc == 3),
                                )
                            ob = pout.tile([128, 512], B16, tag="ob", name="ob")
                            nc.vector.tensor_add(ob[:], pm[:], obas[(k, oh)][:])
                            nc.scalar.dma_start(
                                out=outr[:, ni, 512 * oh:512 * (oh + 1)], in_=ob[:])
                        ua.append(oa_u)
                        ub.append(ob_u)
                return ua, ub

            def run_interleaved(cur, prev):
                # proportional round-robin interleave of cur and prev
                na, nb = len(cur), len(prev)
                ia = ib = 0
                while ia < na or ib < nb:
                    if ib * max(na, 1) * ratio[1] <= ia * max(nb, 1) * ratio[0] and ib < nb or ia >= na:
                        prev[ib](); ib += 1
                    else:
                        cur[ia](); ia += 1

            xss = {0: xs0, 1: xs1}
            atqs = {}
            for sp in range(1, 4):
                atqs[sp - 1] = None
            nb_pend = []
            for sp in range(4):
                cur = list(nb_pend)
                nb_pend = []
                cur += proj_units(sp)
                if sp >= 2:
                    cur = cur + o_units(sp - 2, atqs[sp - 2])
                prev = []
                if sp >= 1:
                    atqs[sp - 1] = patq.tile([128, 4, 512], B16, tag="atq",
                                             name=f"atq{sp - 1}")
                    prev = attn_units(sp - 1, atqs[sp - 1], batched=True)
                run_interleaved(cur, prev)
            # phase 4: attention chunk 3; O(2) rides the first two chains,
            # O(3)'s uc0/1 half rides the last two, its uc2/3 half drains.
            atqs[3] = patq.tile([128, 4, 512], B16, tag="atq", name="atq3")
            o3a, o3b = o_units_split(3, atqs[3])
            run_interleaved(nb_pend + o_units(2, atqs[2]),
                            attn_units(3, atqs[3], hps=(0, 1), batched=True))
            run_interleaved(o3a,
                            attn_units(3, atqs[3], hps=(2, 3), batched=True))
            for u in o3b:
                u()

    nc.compile()
    return nc


_NC_CACHE = {}


def _get_nc(repeat=1, **kw):
    key = (repeat, tuple(sorted(kw.items())))
    if key not in _NC_CACHE:
        _NC_CACHE[key] = _build(repeat, **kw)
    return _NC_CACHE[key]


def _host_inputs(x, Wq, bq, Wk, bk, Wv, bv, Wo, bo):
    """Build the 8 per-core input maps."""
    import ml_dtypes
    f32 = np.float32
    B16 = ml_dtypes.bfloat16

    def rnd(a):
        return np.ascontiguousarray(a, dtype=f32).astype(B16)

    r = np.arange(128)[:, None]
    c = np.arange(128)[None, :]
    mask1 = np.where(r <= c, f32(0.0), f32(-1e4)).astype(f32)
    mask = np.concatenate([mask1, mask1], axis=1)

    in_maps = []
    for core in range(8):
        bi, hg = core // 2, core % 2
        hsl = slice(512 * hg, 512 * (hg + 1))
        # xT swizzled: [ns, p, kc, col] = x[bi].T[kc*128+p, 256*ns+col]
        xTl = rnd(np.ascontiguousarray(
            x[bi].T.reshape(8, 128, 8, 256).transpose(2, 1, 0, 3)))
        wqkTl = rnd(np.ascontiguousarray(
            np.concatenate([Wq[hsl].T, Wk[hsl].T], axis=1).reshape(8, 128, 1024)
            .transpose(1, 0, 2)))
        wvTl = rnd(np.ascontiguousarray(
            Wv[hsl].T.reshape(8, 128, 512).transpose(1, 0, 2)))
        woTl = rnd(np.ascontiguousarray(
            Wo[:, hsl].T.reshape(4, 128, 1024).transpose(1, 0, 2)))
        bq_l, bk_l = bq[hsl], bk[hsl]
        bqk_t = np.stack(
            [bq_l[128 * i:128 * (i + 1)] for i in range(4)]
            + [bk_l[128 * i:128 * (i + 1)] for i in range(4)]
            + [np.ones(128, dtype=f32)] * 8, axis=1
        ).astype(f32)
        bvb_t = np.broadcast_to(bv[hsl].astype(f32), (128, 512)).copy()
        if hg == 0:
            bob_t = np.broadcast_to(bo.astype(f32), (128, 1024)).copy()
        else:
            bob_t = np.zeros((128, 1024), dtype=f32)
        in_maps.append({
            "xT": xTl, "wqkT": wqkTl, "wvT": wvTl, "woT": woTl,
            "bqk": bqk_t, "bvb": bvb_t, "bob": bob_t, "maskt": mask,
        })
    return in_maps


def kernel(x, Wq, bq, Wk, bk, Wv, bv, Wo, bo):
    from concourse.bass_utils import run_bass_kernel_spmd

    x = np.asarray(x); Wq = np.asarray(Wq); bq = np.asarray(bq)
    Wk = np.asarray(Wk); bk = np.asarray(bk); Wv = np.asarray(Wv)
    bv = np.asarray(bv); Wo = np.asarray(Wo); bo = np.asarray(bo)

    nc = _get_nc()
    in_maps = _host_inputs(x, Wq, bq, Wk, bk, Wv, bv, Wo, bo)
    r = run_bass_kernel_spmd(nc, in_maps, list(range(8)))

    out = np.empty((4, 2048, 1024), dtype=np.float32)
    for bi in range(4):
        out[bi] = (r.results[2 * bi]["out"].astype(np.float32)
                   + r.results[2 * bi + 1]["out"].astype(np.float32))
    return out
